# revision 30
# baseline (speedup 1.0000x reference)
"""Trainium2 Bass kernel for nn_CrossAttention_5385888989393.

Contract: kernel(**inputs) takes FULL inputs (batch 8) and returns the FULL
output, sharding batch-parallel across 8 NeuronCores (1 batch element per
core, no collectives).

Algorithm per batch (channel attention, contraction over spatial n=4096):
    G     = f_m @ f_n^T                     [512, 512]  Gram over n
    T2T   = G^T @ Wq^T                      [512, 512]  (G stationary)
    D^T_h = Wk_h-contraction with T2T       [64, 64] per head (diag tiles)
    E^T   = exp(D^T * scale) * headmask     (softmax numerator, transposed)
    SE_h  = E_h @ Wv_h   (via lhsT = E^T)   [64, 512]
    S_h   = SE_h / rowsum(E_h)              (deferred softmax normalization)
    M^T   = S-contraction with Wout^T       [512, 512]
    out   = (M @ f_n) + bout                [512, 4096]

This is ~2x fewer FLOPs than the naive q/k/v-projection path because the
spatial dimension collapses through the Gram matrix immediately.

Matmuls run in float32r mode (full-rate PE, ~1.5e-4 rel err, fp32 storage).
The BIR verifier requires fp32r-matmul inputs to be produced by an op that
rounds to fp32r, so the input staging tiles get explicit rounding copies
(split across DVE/ACT to balance engine load) and PE transposes of rounded
data run in f32r mode (1.5 cyc/row vs 2.0 for fp32). Chunk 0 uses fp32
transposes + DVE rounding so the pipeline starts without waiting on ACT.
"""
import sys

if "/opt/trn_rl_repo" not in sys.path:
    sys.path.insert(0, "/opt/trn_rl_repo")

import numpy as np

import concourse.bass as bass
import concourse.tile as tile
from concourse import bacc, mybir
from concourse.bass_utils import run_bass_kernel_spmd
F32 = mybir.dt.float32
F32R = mybir.dt.float32r
EXP = mybir.ActivationFunctionType.Exp
CP = mybir.ActivationFunctionType.Copy
IDENT_FN = mybir.ActivationFunctionType.Identity

P = 128          # partitions
C = 512          # channels
CT = C // P      # 4 channel tiles
NN = 4096        # spatial (64*64)
NCH = NN // 512  # 8 column chunks of 512
NSUB = NN // P   # 32 column subchunks of 128
DH = 64
SCALE = DH ** -0.5
B = 8            # batch == n_cores

_CACHED_NC = None
_CACHED_RUNNER = None

_IDENT = np.eye(P, dtype=np.float32)
_DMASK = np.kron(np.eye(2, dtype=np.float32), np.ones((DH, DH), np.float32))
_ONES2 = np.ones((P, 2), dtype=np.float32)


def _build():
    nc = bacc.Bacc("TRN2", target_bir_lowering=False, debug=False, num_devices=B)

    fm_d = nc.dram_tensor("f_m", [C, NN], F32, kind="ExternalInput").ap()
    fn_d = nc.dram_tensor("f_n", [C, NN], F32, kind="ExternalInput").ap()
    wqt_d = nc.dram_tensor("WqT", [C, C], F32, kind="ExternalInput").ap()
    wkt_d = nc.dram_tensor("WkT", [C, C], F32, kind="ExternalInput").ap()
    wv_d = nc.dram_tensor("Wv", [C, C], F32, kind="ExternalInput").ap()
    woutt_d = nc.dram_tensor("WoutT", [C, C], F32, kind="ExternalInput").ap()
    bout_d = nc.dram_tensor("bout", [C], F32, kind="ExternalInput").ap()
    ident_d = nc.dram_tensor("ident", [P, P], F32, kind="ExternalInput").ap()
    dmask_d = nc.dram_tensor("dmask", [P, P], F32, kind="ExternalInput").ap()
    ones2_d = nc.dram_tensor("ones2", [P, 2], F32, kind="ExternalInput").ap()
    out_d = nc.dram_tensor("out", [C, NN], F32, kind="ExternalOutput").ap()

    with tile.TileContext(nc) as tc:
        with (
            tc.tile_pool(name="const", bufs=1) as const,
            tc.tile_pool(name="w", bufs=1) as wpool,
            tc.tile_pool(name="wstage", bufs=1) as wstage,
            tc.tile_pool(name="fnst", bufs=2) as fnst,
            tc.tile_pool(name="fmst", bufs=2) as fmst,
            tc.tile_pool(name="fmr", bufs=2) as fmrpool,
            tc.tile_pool(name="ft", bufs=3) as ftpool,
            tc.tile_pool(name="small", bufs=1) as small,
            tc.tile_pool(name="fnr", bufs=1) as fnrpool,
            tc.tile_pool(name="outst", bufs=2) as outst,
            tc.tile_pool(name="gacc", bufs=1, space="PSUM") as gacc,
            tc.tile_pool(name="work", bufs=2, space="PSUM") as work,
        ):
            # ---------- constants (host-provided; avoids slow gpsimd
            # memset/affine_select on the startup path) ----------
            ident = const.tile([P, P], F32, tag="ident")
            nc.sync.dma_start(ident[:], ident_d)
            ident_r = const.tile([P, P], F32R, tag="ident_r")
            nc.vector.tensor_copy(ident_r[:], ident[:])

            ones2 = const.tile([P, 2], F32, tag="ones2")
            nc.sync.dma_start(ones2[:], ones2_d)
            ones2_r = const.tile([P, 2], F32R, tag="ones2_r")
            nc.vector.tensor_copy(ones2_r[:], ones2[:])

            # block-diagonal 0/1 mask to zero cross-head blocks of E^T
            dmask = const.tile([P, P], F32, tag="dmask")
            nc.sync.dma_start(dmask[:], dmask_d)

            # ---------- phase 1: Gram accumulation over 32 subchunks ------
            g_ps = [
                gacc.tile([P, C], F32, tag=f"g{at}", name=f"g_ps{at}")
                for at in range(CT)
            ]

            fnr_res = [[None] * NCH for _ in range(CT)]
            for ch in range(NCH):
                fn_tiles = {}
                fm_tiles = {}
                for ct in range(CT):
                    t = fmst.tile([P, 512], F32, tag=f"fmst{ct}")
                    nc.gpsimd.dma_start(
                        t[:], fm_d[ct * P:(ct + 1) * P, ch * 512:(ch + 1) * 512]
                    )
                    fm_tiles[ct] = t
                    t = fnst.tile([P, 512], F32, tag=f"fnst{ct}")
                    nc.sync.dma_start(
                        t[:], fn_d[ct * P:(ct + 1) * P, ch * 512:(ch + 1) * 512]
                    )
                    fn_tiles[ct] = t

                # round f_n chunk to resident f32r (phase-3 GEMM rhs; also the
                # f32r-transpose source for chunks > 0). DVE for the first
                # chunk so the pipeline start doesn't wait on slow ACT copies.
                for ct in range(CT):
                    r = fnrpool.tile([P, 512], F32R, tag=f"fnr_{ct}_{ch}",
                                     name=f"fnr_{ct}_{ch}")
                    if ct < 2:
                        nc.vector.tensor_copy(r[:], fn_tiles[ct][:])
                    else:
                        nc.scalar.activation(r[:], fn_tiles[ct][:], CP)
                    fnr_res[ct][ch] = r

                fmr_tiles = {}
                for ct in range(CT):
                    r = fmrpool.tile([P, 512], F32R, tag=f"fmr{ct}")
                    nc.scalar.activation(r[:], fm_tiles[ct][:], CP)
                    fmr_tiles[ct] = r

                for su in range(4):
                    s = ch * 4 + su
                    sl = slice(su * P, (su + 1) * P)
                    # transpose f_m subchunk -> [n128, c512] (f32r mode)
                    tpm = work.tile([P, C], F32R, tag="wk0", name="tpm")
                    for ct in range(CT):
                        nc.tensor.transpose(
                            tpm[:, ct * P:(ct + 1) * P],
                            fmr_tiles[ct][:, sl], ident_r[:]
                        )
                    fmT = ftpool.tile([P, C], F32R, tag="fmT")
                    nc.vector.tensor_copy(fmT[:], tpm[:])

                    # transpose f_n subchunk (f32r mode)
                    tpn = work.tile([P, C], F32R, tag="wk1", name="tpn")
                    for ct in range(CT):
                        nc.tensor.transpose(
                            tpn[:, ct * P:(ct + 1) * P],
                            fnr_res[ct][ch][:, sl], ident_r[:]
                        )
                    fnT = ftpool.tile([P, C], F32R, tag="fnT")
                    nc.vector.tensor_copy(fnT[:], tpn[:])

                    # Gram: G[a-tile, :] += fmT[:, a-tile].T @ fnT
                    for at in range(CT):
                        nc.tensor.matmul(
                            g_ps[at][:],
                            fmT[:, at * P:(at + 1) * P],
                            fnT[:],
                            start=(s == 0),
                            stop=(s == NSUB - 1),
                        )

            # ---------- weights (needed from phase 2; loaded during phase 1
            # DMA slack) + bout ----------
            def load_rounded(dram_rows, name):
                res = []
                for rt in range(CT):
                    st = wstage.tile([P, C], F32, tag=f"wnat{rt}")
                    nc.sync.dma_start(st[:], dram_rows[rt * P:(rt + 1) * P, :])
                    rs = wpool.tile([P, C], F32R, tag=f"{name}{rt}")
                    nc.scalar.activation(rs[:], st[:], CP)
                    res.append(rs)
                return res

            WqT = load_rounded(wqt_d, "wqT")      # WqT[a][., (h,i)]
            WkT = load_rounded(wkt_d, "wkT")      # WkT[b][., (h,j)]
            WoutT = load_rounded(woutt_d, "woT")  # WoutT[e][., o]
            Wv_r = load_rounded(wv_d, "wv")       # Wv rows (h,j), cols c

            bout_sb = []
            bview = bout_d.rearrange("(t p) -> t p", p=P)
            for ct in range(CT):
                bt = const.tile([P, 1], F32, tag=f"bout{ct}")
                nc.sync.dma_start(bt[:], bview[ct].unsqueeze(1))
                bout_sb.append(bt)

            G_sb = []
            for at in range(CT):
                g = small.tile([P, C], F32R, tag=f"G{at}")
                nc.vector.tensor_copy(g[:], g_ps[at][:])
                G_sb.append(g)

            # ---------- phase 2: logits, softmax, value mixing ------------
            # T2T[b, (h,i)] = sum_a G[a, b] * WqT[a, (h,i)]
            # (G natural as stationary -> transposed product for free)
            T2T_sb = []
            for bt in range(CT):
                ps = work.tile([P, C], F32, tag="wk1", name="t2tps")
                for at in range(CT):
                    nc.tensor.matmul(
                        ps[:],
                        G_sb[at][:, bt * P:(bt + 1) * P],
                        WqT[at][:],
                        start=(at == 0),
                        stop=(at == CT - 1),
                    )
                t = small.tile([P, C], F32R, tag=f"T2T_{bt}")
                nc.vector.tensor_copy(t[:], ps[:])
                T2T_sb.append(t)

            # Diagonal head-pair tiles of D^T = Wk @ T2T ; E^T = exp(scale*D^T)
            ET = []
            for jt in range(CT):
                sl = slice(jt * P, (jt + 1) * P)
                ps = work.tile([P, P], F32, tag="wk0", name="dps")
                for bt in range(CT):
                    nc.tensor.matmul(
                        ps[:], WkT[bt][:, sl], T2T_sb[bt][:, sl],
                        start=(bt == 0), stop=(bt == CT - 1),
                    )
                etmp = small.tile([P, P], F32, tag="etmp")
                nc.scalar.activation(etmp[:], ps[:], EXP, scale=SCALE)
                e = small.tile([P, P], F32R, tag=f"G{jt}", name=f"ET{jt}")
                # zero the cross-head blocks so full-width matmuls (SE,
                # rowsums) see exact per-head separation
                nc.vector.tensor_mul(e[:], etmp[:], dmask[:])
                ET.append(e)

            # rowsums r[(h,i)] = sum_j E_h[i, j]
            inv_sb = []
            for it in range(CT):
                rps = work.tile([P, 2], F32, tag="wk1", name="rps")
                nc.tensor.matmul(rps[:], ET[it][:], ones2_r[:], start=True,
                                 stop=True)
                inv = small.tile([P, 1], F32, tag=f"inv{it}")
                nc.vector.reciprocal(inv[:], rps[:, 0:1])
                inv_sb.append(inv)

            # SE_h = E_h @ Wv_h ; S = SE * inv_r (deferred softmax division)
            S_sb = []
            for it in range(CT):
                seps = work.tile([P, C], F32, tag="wk0", name="seps")
                nc.tensor.matmul(
                    seps[:], ET[it][:], Wv_r[it][:], start=True, stop=True,
                )
                s_t = small.tile([P, C], F32R, tag=f"S{it}", name=f"S{it}")
                nc.vector.tensor_scalar_mul(s_t[:], seps[:], inv_sb[it][:])
                S_sb.append(s_t)

            # M^T[c, o] = sum_e S[e][:, c] * WoutT[e][:, o]
            MT_sb = []
            for ct in range(CT):
                ps = work.tile([P, C], F32, tag="wk1", name="mtps")
                for et in range(CT):
                    nc.tensor.matmul(
                        ps[:],
                        S_sb[et][:, ct * P:(ct + 1) * P],
                        WoutT[et][:],
                        start=(et == 0),
                        stop=(et == CT - 1),
                    )
                t = small.tile([P, C], F32R, tag=f"T2T_{ct}", name=f"MT{ct}")
                nc.vector.tensor_copy(t[:], ps[:])
                MT_sb.append(t)

            # ---------- phase 3: out = M @ f_n + bout ----------------------
            for ch in range(NCH):
                fnr = [fnr_res[ct][ch] for ct in range(CT)]
                for ot in range(CT):
                    ps = gacc.tile([P, 512], F32, tag=f"g{ot}", name=f"ops{ot}")
                    for ct in range(CT):
                        nc.tensor.matmul(
                            ps[:],
                            MT_sb[ct][:, ot * P:(ot + 1) * P],
                            fnr[ct][:],
                            start=(ct == 0),
                            stop=(ct == CT - 1),
                        )
                    o = outst.tile([P, 512], F32, tag=f"out{ot}")
                    # ACT helps mid-stream; keep the last chunk all on DVE so
                    # the tail drains fast
                    if ot >= 2 and ch < NCH - 1:
                        nc.scalar.activation(o[:], ps[:], IDENT_FN,
                                             bias=bout_sb[ot][:])
                    else:
                        nc.vector.tensor_scalar_add(o[:], ps[:], bout_sb[ot][:])
                    nc.sync.dma_start(
                        out_d[ot * P:(ot + 1) * P, ch * 512:(ch + 1) * 512], o[:]
                    )

    nc.compile()
    return nc


def _get_nc():
    global _CACHED_NC
    if _CACHED_NC is None:
        _CACHED_NC = _build()
    return _CACHED_NC


def _get_runner():
    """Memoized PJRT runner: jax.jit-compiled once, reused across kernel()
    calls (run_bass_kernel_spmd rebuilds the jit closure every call, which
    forces a ~minute-long recompile)."""
    global _CACHED_RUNNER
    if _CACHED_RUNNER is not None:
        return _CACHED_RUNNER

    import jax
    from jax.sharding import Mesh, PartitionSpec
    from jax.experimental.shard_map import shard_map
    import concourse.mybir as mybir_
    from concourse.bass2jax import (
        _bass_exec_p,
        install_neuronx_cc_hook,
        partition_id_tensor,
    )

    nc = _get_nc()
    install_neuronx_cc_hook()

    partition_name = (
        nc.partition_id_tensor.name if nc.partition_id_tensor else None
    )
    in_names = []
    out_names = []
    out_avals = []
    out_shapes = []
    for alloc in nc.m.functions[0].allocations:
        if not isinstance(alloc, mybir_.MemoryLocationSet):
            continue
        name = alloc.memorylocations[0].name
        if alloc.kind == "ExternalInput":
            if name != partition_name:
                in_names.append(name)
        elif alloc.kind == "ExternalOutput":
            shape = tuple(alloc.tensor_shape)
            dtype = mybir_.dt.np(alloc.dtype)
            out_names.append(name)
            out_avals.append(jax.core.ShapedArray(shape, dtype))
            out_shapes.append((shape, dtype))
    n_params = len(in_names)
    n_outs = len(out_names)
    all_names = tuple(in_names + out_names)
    if partition_name is not None:
        all_names = all_names + (partition_name,)
    donate = tuple(range(n_params, n_params + n_outs))

    def _body(*args):
        operands = list(args)
        if partition_name is not None:
            operands.append(partition_id_tensor())
        outs = _bass_exec_p.bind(
            *operands,
            out_avals=tuple(out_avals),
            in_names=all_names,
            out_names=tuple(out_names),
            lowering_input_output_aliases=(),
            sim_require_finite=True,
            sim_require_nnan=True,
            nc=nc,
        )
        return tuple(outs)

    devices = jax.devices()[:B]
    mesh = Mesh(np.asarray(devices), ("core",))
    sharded = jax.jit(
        shard_map(
            _body,
            mesh=mesh,
            in_specs=(PartitionSpec("core"),) * (n_params + n_outs),
            out_specs=(PartitionSpec("core"),) * n_outs,
            check_rep=False,
        ),
        donate_argnums=donate,
        keep_unused=True,
    )

    def run(in_maps):
        concat_in = [
            np.concatenate([np.asarray(m[k]) for m in in_maps], axis=0)
            for k in in_names
        ]
        concat_zeros = [
            np.zeros((B * s[0], *s[1:]), dt) for (s, dt) in out_shapes
        ]
        out_arrs = sharded(*concat_in, *concat_zeros)
        return [
            {
                k: np.asarray(out_arrs[i]).reshape(B, *out_shapes[i][0])[c]
                for i, k in enumerate(out_names)
            }
            for c in range(B)
        ]

    _CACHED_RUNNER = run
    return run


def kernel(f_m, f_n, Wq, Wkv, Wout, bout, trace=False):
    f_m = np.ascontiguousarray(np.asarray(f_m, dtype=np.float32))
    f_n = np.ascontiguousarray(np.asarray(f_n, dtype=np.float32))
    Wq = np.ascontiguousarray(np.asarray(Wq, dtype=np.float32))
    Wkv = np.ascontiguousarray(np.asarray(Wkv, dtype=np.float32))
    Wout = np.ascontiguousarray(np.asarray(Wout, dtype=np.float32))
    bout = np.ascontiguousarray(np.asarray(bout, dtype=np.float32))

    b, c, h, w = f_m.shape
    nc = _get_nc()
    wqt = np.ascontiguousarray(Wq.T)
    wkt = np.ascontiguousarray(Wkv[:C].T)
    wv = np.ascontiguousarray(Wkv[C:])
    woutt = np.ascontiguousarray(Wout.T)
    in_maps = [
        {
            "f_m": f_m[i].reshape(C, NN),
            "f_n": f_n[i].reshape(C, NN),
            "WqT": wqt,
            "WkT": wkt,
            "Wv": wv,
            "WoutT": woutt,
            "bout": bout,
            "ident": _IDENT,
            "dmask": _DMASK,
            "ones2": _ONES2,
        }
        for i in range(b)
    ]
    if trace:
        res = run_bass_kernel_spmd(
            nc, in_maps, core_ids=list(range(B)), trace=True
        )
        kernel.last_results = res
        results = res.results
    else:
        results = _get_runner()(in_maps)
    return np.stack([r["out"].reshape(c, h, w) for r in results])


# revision 31
# speedup vs baseline: 1.0135x; 1.0135x over previous
"""Trainium2 Bass kernel for nn_CrossAttention_5385888989393.

Contract: kernel(**inputs) takes FULL inputs (batch 8) and returns the FULL
output, sharding batch-parallel across 8 NeuronCores (1 batch element per
core, no collectives).

Algorithm per batch (channel attention, contraction over spatial n=4096):
    G     = f_m @ f_n^T                     [512, 512]  Gram over n
    T2T   = G^T @ Wq^T                      [512, 512]  (G stationary)
    D^T_h = Wk_h-contraction with T2T       [64, 64] per head (diag tiles)
    E^T   = exp(D^T * scale) * headmask     (softmax numerator, transposed)
    SE_h  = E_h @ Wv_h   (via lhsT = E^T)   [64, 512]
    S_h   = SE_h / rowsum(E_h)              (deferred softmax normalization)
    M^T   = S-contraction with Wout^T       [512, 512]
    out   = (M @ f_n) + bout                [512, 4096]

This is ~2x fewer FLOPs than the naive q/k/v-projection path because the
spatial dimension collapses through the Gram matrix immediately.

Matmuls run in float32r mode (full-rate PE, ~1.5e-4 rel err, fp32 storage).
The BIR verifier requires fp32r-matmul inputs to be produced by an op that
rounds to fp32r, so the input staging tiles get explicit rounding copies
(split across DVE/ACT to balance engine load) and PE transposes of rounded
data run in f32r mode (1.5 cyc/row vs 2.0 for fp32). Chunk 0 uses fp32
transposes + DVE rounding so the pipeline starts without waiting on ACT.
"""
import sys

if "/opt/trn_rl_repo" not in sys.path:
    sys.path.insert(0, "/opt/trn_rl_repo")

import numpy as np

import concourse.bass as bass
import concourse.tile as tile
from concourse import bacc, mybir
from concourse.bass_utils import run_bass_kernel_spmd
F32 = mybir.dt.float32
F32R = mybir.dt.float32r
EXP = mybir.ActivationFunctionType.Exp
CP = mybir.ActivationFunctionType.Copy
IDENT_FN = mybir.ActivationFunctionType.Identity

P = 128          # partitions
C = 512          # channels
CT = C // P      # 4 channel tiles
NN = 4096        # spatial (64*64)
NCH = NN // 512  # 8 column chunks of 512
NSUB = NN // P   # 32 column subchunks of 128
DH = 64
SCALE = DH ** -0.5
B = 8            # batch == n_cores

_CACHED_NC = None
_CACHED_RUNNER = None

_IDENT = np.eye(P, dtype=np.float32)
_DMASK = np.kron(np.eye(2, dtype=np.float32), np.ones((DH, DH), np.float32))
_ONES2 = np.ones((P, 2), dtype=np.float32)


def _build():
    nc = bacc.Bacc("TRN2", target_bir_lowering=False, debug=False, num_devices=B)

    fm_d = nc.dram_tensor("f_m", [C, NN], F32, kind="ExternalInput").ap()
    fn_d = nc.dram_tensor("f_n", [C, NN], F32, kind="ExternalInput").ap()
    wqt_d = nc.dram_tensor("WqT", [C, C], F32, kind="ExternalInput").ap()
    wkt_d = nc.dram_tensor("WkT", [C, C], F32, kind="ExternalInput").ap()
    wv_d = nc.dram_tensor("Wv", [C, C], F32, kind="ExternalInput").ap()
    woutt_d = nc.dram_tensor("WoutT", [C, C], F32, kind="ExternalInput").ap()
    bout_d = nc.dram_tensor("bout", [C], F32, kind="ExternalInput").ap()
    ident_d = nc.dram_tensor("ident", [P, P], F32, kind="ExternalInput").ap()
    dmask_d = nc.dram_tensor("dmask", [P, P], F32, kind="ExternalInput").ap()
    ones2_d = nc.dram_tensor("ones2", [P, 2], F32, kind="ExternalInput").ap()
    out_d = nc.dram_tensor("out", [C, NN], F32, kind="ExternalOutput").ap()

    with tile.TileContext(nc) as tc:
        with (
            tc.tile_pool(name="const", bufs=1) as const,
            tc.tile_pool(name="w", bufs=1) as wpool,
            tc.tile_pool(name="wstage", bufs=1) as wstage,
            tc.tile_pool(name="fnst", bufs=2) as fnst,
            tc.tile_pool(name="fmst", bufs=2) as fmst,
            tc.tile_pool(name="fmr", bufs=2) as fmrpool,
            tc.tile_pool(name="ft", bufs=3) as ftpool,
            tc.tile_pool(name="small", bufs=1) as small,
            tc.tile_pool(name="fnr", bufs=1) as fnrpool,
            tc.tile_pool(name="outst", bufs=2) as outst,
            tc.tile_pool(name="gacc", bufs=1, space="PSUM") as gacc,
            tc.tile_pool(name="work", bufs=2, space="PSUM") as work,
        ):
            # ---------- constants (host-provided; avoids slow gpsimd
            # memset/affine_select on the startup path) ----------
            ident = const.tile([P, P], F32, tag="ident")
            nc.sync.dma_start(ident[:], ident_d)
            ident_r = const.tile([P, P], F32R, tag="ident_r")
            nc.vector.tensor_copy(ident_r[:], ident[:])

            ones2 = const.tile([P, 2], F32, tag="ones2")
            nc.sync.dma_start(ones2[:], ones2_d)
            ones2_r = const.tile([P, 2], F32R, tag="ones2_r")
            nc.vector.tensor_copy(ones2_r[:], ones2[:])

            # block-diagonal 0/1 mask to zero cross-head blocks of E^T
            dmask = const.tile([P, P], F32, tag="dmask")
            nc.sync.dma_start(dmask[:], dmask_d)

            # ---------- phase 1: Gram accumulation over 32 subchunks ------
            g_ps = [
                gacc.tile([P, C], F32, tag=f"g{at}", name=f"g_ps{at}")
                for at in range(CT)
            ]

            fnr_res = [[None] * NCH for _ in range(CT)]
            for ch in range(NCH):
                fn_tiles = {}
                fm_tiles = {}
                for ct in range(CT):
                    t = fmst.tile([P, 512], F32, tag=f"fmst{ct}")
                    nc.sync.dma_start(
                        t[:], fm_d[ct * P:(ct + 1) * P, ch * 512:(ch + 1) * 512]
                    )
                    fm_tiles[ct] = t
                    t = fnst.tile([P, 512], F32, tag=f"fnst{ct}")
                    nc.sync.dma_start(
                        t[:], fn_d[ct * P:(ct + 1) * P, ch * 512:(ch + 1) * 512]
                    )
                    fn_tiles[ct] = t

                # round f_n chunk to resident f32r (phase-3 GEMM rhs; also the
                # f32r-transpose source for chunks > 0). DVE for the first
                # chunk so the pipeline start doesn't wait on slow ACT copies.
                for ct in range(CT):
                    r = fnrpool.tile([P, 512], F32R, tag=f"fnr_{ct}_{ch}",
                                     name=f"fnr_{ct}_{ch}")
                    if ct < 2:
                        nc.vector.tensor_copy(r[:], fn_tiles[ct][:])
                    else:
                        nc.scalar.activation(r[:], fn_tiles[ct][:], CP)
                    fnr_res[ct][ch] = r

                fmr_tiles = {}
                for ct in range(CT):
                    r = fmrpool.tile([P, 512], F32R, tag=f"fmr{ct}")
                    nc.scalar.activation(r[:], fm_tiles[ct][:], CP)
                    fmr_tiles[ct] = r

                for su in range(4):
                    s = ch * 4 + su
                    sl = slice(su * P, (su + 1) * P)
                    # transpose f_m subchunk -> [n128, c512] (f32r mode)
                    tpm = work.tile([P, C], F32R, tag="wk0", name="tpm")
                    for ct in range(CT):
                        nc.tensor.transpose(
                            tpm[:, ct * P:(ct + 1) * P],
                            fmr_tiles[ct][:, sl], ident_r[:]
                        )
                    fmT = ftpool.tile([P, C], F32R, tag="fmT")
                    nc.vector.tensor_copy(fmT[:], tpm[:])

                    # transpose f_n subchunk (f32r mode)
                    tpn = work.tile([P, C], F32R, tag="wk1", name="tpn")
                    for ct in range(CT):
                        nc.tensor.transpose(
                            tpn[:, ct * P:(ct + 1) * P],
                            fnr_res[ct][ch][:, sl], ident_r[:]
                        )
                    fnT = ftpool.tile([P, C], F32R, tag="fnT")
                    nc.vector.tensor_copy(fnT[:], tpn[:])

                    # Gram: G[a-tile, :] += fmT[:, a-tile].T @ fnT
                    for at in range(CT):
                        nc.tensor.matmul(
                            g_ps[at][:],
                            fmT[:, at * P:(at + 1) * P],
                            fnT[:],
                            start=(s == 0),
                            stop=(s == NSUB - 1),
                        )

            # ---------- weights (needed from phase 2; loaded during phase 1
            # DMA slack) + bout ----------
            def load_rounded(dram_rows, name):
                res = []
                for rt in range(CT):
                    st = wstage.tile([P, C], F32, tag=f"wnat{rt}")
                    nc.sync.dma_start(st[:], dram_rows[rt * P:(rt + 1) * P, :])
                    rs = wpool.tile([P, C], F32R, tag=f"{name}{rt}")
                    nc.scalar.activation(rs[:], st[:], CP)
                    res.append(rs)
                return res

            WqT = load_rounded(wqt_d, "wqT")      # WqT[a][., (h,i)]
            WkT = load_rounded(wkt_d, "wkT")      # WkT[b][., (h,j)]
            WoutT = load_rounded(woutt_d, "woT")  # WoutT[e][., o]
            Wv_r = load_rounded(wv_d, "wv")       # Wv rows (h,j), cols c

            bout_sb = []
            bview = bout_d.rearrange("(t p) -> t p", p=P)
            for ct in range(CT):
                bt = const.tile([P, 1], F32, tag=f"bout{ct}")
                nc.sync.dma_start(bt[:], bview[ct].unsqueeze(1))
                bout_sb.append(bt)

            G_sb = []
            for at in range(CT):
                g = small.tile([P, C], F32R, tag=f"G{at}")
                nc.vector.tensor_copy(g[:], g_ps[at][:])
                G_sb.append(g)

            # ---------- phase 2: logits, softmax, value mixing ------------
            # T2T[b, (h,i)] = sum_a G[a, b] * WqT[a, (h,i)]
            # (G natural as stationary -> transposed product for free)
            T2T_sb = []
            for bt in range(CT):
                ps = work.tile([P, C], F32, tag="wk1", name="t2tps")
                for at in range(CT):
                    nc.tensor.matmul(
                        ps[:],
                        G_sb[at][:, bt * P:(bt + 1) * P],
                        WqT[at][:],
                        start=(at == 0),
                        stop=(at == CT - 1),
                    )
                t = small.tile([P, C], F32R, tag=f"T2T_{bt}")
                nc.vector.tensor_copy(t[:], ps[:])
                T2T_sb.append(t)

            # Diagonal head-pair tiles of D^T = Wk @ T2T ; E^T = exp(scale*D^T)
            ET = []
            for jt in range(CT):
                sl = slice(jt * P, (jt + 1) * P)
                ps = work.tile([P, P], F32, tag="wk0", name="dps")
                for bt in range(CT):
                    nc.tensor.matmul(
                        ps[:], WkT[bt][:, sl], T2T_sb[bt][:, sl],
                        start=(bt == 0), stop=(bt == CT - 1),
                    )
                etmp = small.tile([P, P], F32, tag="etmp")
                nc.scalar.activation(etmp[:], ps[:], EXP, scale=SCALE)
                e = small.tile([P, P], F32R, tag=f"G{jt}", name=f"ET{jt}")
                # zero the cross-head blocks so full-width matmuls (SE,
                # rowsums) see exact per-head separation
                nc.vector.tensor_mul(e[:], etmp[:], dmask[:])
                ET.append(e)

            # rowsums r[(h,i)] = sum_j E_h[i, j]
            inv_sb = []
            for it in range(CT):
                rps = work.tile([P, 2], F32, tag="wk1", name="rps")
                nc.tensor.matmul(rps[:], ET[it][:], ones2_r[:], start=True,
                                 stop=True)
                inv = small.tile([P, 1], F32, tag=f"inv{it}")
                nc.vector.reciprocal(inv[:], rps[:, 0:1])
                inv_sb.append(inv)

            # SE_h = E_h @ Wv_h ; S = SE * inv_r (deferred softmax division)
            S_sb = []
            for it in range(CT):
                seps = work.tile([P, C], F32, tag="wk0", name="seps")
                nc.tensor.matmul(
                    seps[:], ET[it][:], Wv_r[it][:], start=True, stop=True,
                )
                s_t = small.tile([P, C], F32R, tag=f"S{it}", name=f"S{it}")
                nc.vector.tensor_scalar_mul(s_t[:], seps[:], inv_sb[it][:])
                S_sb.append(s_t)

            # M^T[c, o] = sum_e S[e][:, c] * WoutT[e][:, o]
            MT_sb = []
            for ct in range(CT):
                ps = work.tile([P, C], F32, tag="wk1", name="mtps")
                for et in range(CT):
                    nc.tensor.matmul(
                        ps[:],
                        S_sb[et][:, ct * P:(ct + 1) * P],
                        WoutT[et][:],
                        start=(et == 0),
                        stop=(et == CT - 1),
                    )
                t = small.tile([P, C], F32R, tag=f"T2T_{ct}", name=f"MT{ct}")
                nc.vector.tensor_copy(t[:], ps[:])
                MT_sb.append(t)

            # ---------- phase 3: out = M @ f_n + bout ----------------------
            for ch in range(NCH):
                fnr = [fnr_res[ct][ch] for ct in range(CT)]
                for ot in range(CT):
                    ps = gacc.tile([P, 512], F32, tag=f"g{ot}", name=f"ops{ot}")
                    for ct in range(CT):
                        nc.tensor.matmul(
                            ps[:],
                            MT_sb[ct][:, ot * P:(ot + 1) * P],
                            fnr[ct][:],
                            start=(ct == 0),
                            stop=(ct == CT - 1),
                        )
                    o = outst.tile([P, 512], F32, tag=f"out{ot}")
                    # ACT helps mid-stream; keep the last chunk all on DVE so
                    # the tail drains fast
                    if ot >= 2 and ch < NCH - 1:
                        nc.scalar.activation(o[:], ps[:], IDENT_FN,
                                             bias=bout_sb[ot][:])
                    else:
                        nc.vector.tensor_scalar_add(o[:], ps[:], bout_sb[ot][:])
                    nc.gpsimd.dma_start(
                        out_d[ot * P:(ot + 1) * P, ch * 512:(ch + 1) * 512], o[:]
                    )

    nc.compile()
    return nc


def _get_nc():
    global _CACHED_NC
    if _CACHED_NC is None:
        _CACHED_NC = _build()
    return _CACHED_NC


def _get_runner():
    """Memoized PJRT runner: jax.jit-compiled once, reused across kernel()
    calls (run_bass_kernel_spmd rebuilds the jit closure every call, which
    forces a ~minute-long recompile)."""
    global _CACHED_RUNNER
    if _CACHED_RUNNER is not None:
        return _CACHED_RUNNER

    import jax
    from jax.sharding import Mesh, PartitionSpec
    from jax.experimental.shard_map import shard_map
    import concourse.mybir as mybir_
    from concourse.bass2jax import (
        _bass_exec_p,
        install_neuronx_cc_hook,
        partition_id_tensor,
    )

    nc = _get_nc()
    install_neuronx_cc_hook()

    partition_name = (
        nc.partition_id_tensor.name if nc.partition_id_tensor else None
    )
    in_names = []
    out_names = []
    out_avals = []
    out_shapes = []
    for alloc in nc.m.functions[0].allocations:
        if not isinstance(alloc, mybir_.MemoryLocationSet):
            continue
        name = alloc.memorylocations[0].name
        if alloc.kind == "ExternalInput":
            if name != partition_name:
                in_names.append(name)
        elif alloc.kind == "ExternalOutput":
            shape = tuple(alloc.tensor_shape)
            dtype = mybir_.dt.np(alloc.dtype)
            out_names.append(name)
            out_avals.append(jax.core.ShapedArray(shape, dtype))
            out_shapes.append((shape, dtype))
    n_params = len(in_names)
    n_outs = len(out_names)
    all_names = tuple(in_names + out_names)
    if partition_name is not None:
        all_names = all_names + (partition_name,)
    donate = tuple(range(n_params, n_params + n_outs))

    def _body(*args):
        operands = list(args)
        if partition_name is not None:
            operands.append(partition_id_tensor())
        outs = _bass_exec_p.bind(
            *operands,
            out_avals=tuple(out_avals),
            in_names=all_names,
            out_names=tuple(out_names),
            lowering_input_output_aliases=(),
            sim_require_finite=True,
            sim_require_nnan=True,
            nc=nc,
        )
        return tuple(outs)

    devices = jax.devices()[:B]
    mesh = Mesh(np.asarray(devices), ("core",))
    sharded = jax.jit(
        shard_map(
            _body,
            mesh=mesh,
            in_specs=(PartitionSpec("core"),) * (n_params + n_outs),
            out_specs=(PartitionSpec("core"),) * n_outs,
            check_rep=False,
        ),
        donate_argnums=donate,
        keep_unused=True,
    )

    def run(in_maps):
        concat_in = [
            np.concatenate([np.asarray(m[k]) for m in in_maps], axis=0)
            for k in in_names
        ]
        concat_zeros = [
            np.zeros((B * s[0], *s[1:]), dt) for (s, dt) in out_shapes
        ]
        out_arrs = sharded(*concat_in, *concat_zeros)
        return [
            {
                k: np.asarray(out_arrs[i]).reshape(B, *out_shapes[i][0])[c]
                for i, k in enumerate(out_names)
            }
            for c in range(B)
        ]

    _CACHED_RUNNER = run
    return run


def kernel(f_m, f_n, Wq, Wkv, Wout, bout, trace=False):
    f_m = np.ascontiguousarray(np.asarray(f_m, dtype=np.float32))
    f_n = np.ascontiguousarray(np.asarray(f_n, dtype=np.float32))
    Wq = np.ascontiguousarray(np.asarray(Wq, dtype=np.float32))
    Wkv = np.ascontiguousarray(np.asarray(Wkv, dtype=np.float32))
    Wout = np.ascontiguousarray(np.asarray(Wout, dtype=np.float32))
    bout = np.ascontiguousarray(np.asarray(bout, dtype=np.float32))

    b, c, h, w = f_m.shape
    nc = _get_nc()
    wqt = np.ascontiguousarray(Wq.T)
    wkt = np.ascontiguousarray(Wkv[:C].T)
    wv = np.ascontiguousarray(Wkv[C:])
    woutt = np.ascontiguousarray(Wout.T)
    in_maps = [
        {
            "f_m": f_m[i].reshape(C, NN),
            "f_n": f_n[i].reshape(C, NN),
            "WqT": wqt,
            "WkT": wkt,
            "Wv": wv,
            "WoutT": woutt,
            "bout": bout,
            "ident": _IDENT,
            "dmask": _DMASK,
            "ones2": _ONES2,
        }
        for i in range(b)
    ]
    if trace:
        res = run_bass_kernel_spmd(
            nc, in_maps, core_ids=list(range(B)), trace=True
        )
        kernel.last_results = res
        results = res.results
    else:
        results = _get_runner()(in_maps)
    return np.stack([r["out"].reshape(c, h, w) for r in results])


# revision 33
# speedup vs baseline: 1.0633x; 1.0491x over previous
"""Trainium2 Bass kernel for nn_CrossAttention_5385888989393.

Contract: kernel(**inputs) takes FULL inputs (batch 8) and returns the FULL
output, sharding batch-parallel across 8 NeuronCores (1 batch element per
core, no collectives).

Algorithm per batch (channel attention, contraction over spatial n=4096):
    G     = f_m @ f_n^T                     [512, 512]  Gram over n
    T2T   = G^T @ Wq^T                      [512, 512]  (G stationary)
    D^T_h = Wk_h-contraction with T2T       [64, 64] per head (diag tiles)
    E^T   = exp(D^T * scale) * headmask     (softmax numerator, transposed)
    SE_h  = E_h @ Wv_h   (via lhsT = E^T)   [64, 512]
    S_h   = SE_h / rowsum(E_h)              (deferred softmax normalization)
    M^T   = S-contraction with Wout^T       [512, 512]
    out   = (M @ f_n) + bout                [512, 4096]

This is ~2x fewer FLOPs than the naive q/k/v-projection path because the
spatial dimension collapses through the Gram matrix immediately.

Matmuls run in float32r mode (full-rate PE, ~1.5e-4 rel err, fp32 storage).
The BIR verifier requires fp32r-matmul inputs to be produced by an op that
rounds to fp32r, so the input staging tiles get explicit rounding copies
(split across DVE/ACT to balance engine load) and PE transposes of rounded
data run in f32r mode (1.5 cyc/row vs 2.0 for fp32). Chunk 0 uses fp32
transposes + DVE rounding so the pipeline starts without waiting on ACT.
"""
import sys

if "/opt/trn_rl_repo" not in sys.path:
    sys.path.insert(0, "/opt/trn_rl_repo")

import numpy as np

import concourse.bass as bass
import concourse.tile as tile
from concourse import bacc, mybir
from concourse.bass_utils import run_bass_kernel_spmd
F32 = mybir.dt.float32
F32R = mybir.dt.float32r
EXP = mybir.ActivationFunctionType.Exp
CP = mybir.ActivationFunctionType.Copy
IDENT_FN = mybir.ActivationFunctionType.Identity

P = 128          # partitions
C = 512          # channels
CT = C // P      # 4 channel tiles
NN = 4096        # spatial (64*64)
NCH = NN // 512  # 8 column chunks of 512
NSUB = NN // P   # 32 column subchunks of 128
DH = 64
SCALE = DH ** -0.5
B = 8            # batch == n_cores

_CACHED_NC = None
_CACHED_RUNNER = None

_IDENT = np.eye(P, dtype=np.float32)
_DMASK = np.kron(np.eye(2, dtype=np.float32), np.ones((DH, DH), np.float32))
_ONES2 = np.ones((P, 2), dtype=np.float32)


def _build():
    nc = bacc.Bacc("TRN2", target_bir_lowering=False, debug=False, num_devices=B)

    fm_d = nc.dram_tensor("f_m", [C, NN], F32, kind="ExternalInput").ap()
    fn_d = nc.dram_tensor("f_n", [C, NN], F32, kind="ExternalInput").ap()
    wqt_d = nc.dram_tensor("WqT", [C, C], F32, kind="ExternalInput").ap()
    wkt_d = nc.dram_tensor("WkT", [C, C], F32, kind="ExternalInput").ap()
    wv_d = nc.dram_tensor("Wv", [C, C], F32, kind="ExternalInput").ap()
    woutt_d = nc.dram_tensor("WoutT", [C, C], F32, kind="ExternalInput").ap()
    bout_d = nc.dram_tensor("bout", [C], F32, kind="ExternalInput").ap()
    ident_d = nc.dram_tensor("ident", [P, P], F32, kind="ExternalInput").ap()
    dmask_d = nc.dram_tensor("dmask", [P, P], F32, kind="ExternalInput").ap()
    ones2_d = nc.dram_tensor("ones2", [P, 2], F32, kind="ExternalInput").ap()
    out_d = nc.dram_tensor("out", [C, NN], F32, kind="ExternalOutput").ap()

    with tile.TileContext(nc) as tc:
        with (
            tc.tile_pool(name="const", bufs=1) as const,
            tc.tile_pool(name="w", bufs=1) as wpool,
            tc.tile_pool(name="wstage", bufs=1) as wstage,
            tc.tile_pool(name="fnst", bufs=2) as fnst,
            tc.tile_pool(name="fmst", bufs=2) as fmst,
            tc.tile_pool(name="fmr", bufs=2) as fmrpool,
            tc.tile_pool(name="ft", bufs=3) as ftpool,
            tc.tile_pool(name="small", bufs=1) as small,
            tc.tile_pool(name="fnr", bufs=1) as fnrpool,
            tc.tile_pool(name="outst", bufs=2) as outst,
            tc.tile_pool(name="gacc", bufs=1, space="PSUM") as gacc,
            tc.tile_pool(name="work", bufs=2, space="PSUM") as work,
        ):
            # ---------- constants (host-provided; avoids slow gpsimd
            # memset/affine_select on the startup path) ----------
            ident = const.tile([P, P], F32, tag="ident")
            nc.sync.dma_start(ident[:], ident_d)
            ident_r = const.tile([P, P], F32R, tag="ident_r")
            nc.vector.tensor_copy(ident_r[:], ident[:])

            ones2 = const.tile([P, 2], F32, tag="ones2")
            nc.sync.dma_start(ones2[:], ones2_d)
            ones2_r = const.tile([P, 2], F32R, tag="ones2_r")
            nc.vector.tensor_copy(ones2_r[:], ones2[:])

            # block-diagonal 0/1 mask to zero cross-head blocks of E^T
            dmask = const.tile([P, P], F32, tag="dmask")
            nc.sync.dma_start(dmask[:], dmask_d)

            # ---------- phase 1: Gram accumulation over 32 subchunks ------
            g_ps = [
                gacc.tile([P, C], F32, tag=f"g{at}", name=f"g_ps{at}")
                for at in range(CT)
            ]

            fnr_res = [[None] * NCH for _ in range(CT)]
            for ch in range(NCH):
                fn_tiles = {}
                fm_tiles = {}
                for ct in range(CT):
                    t = fmst.tile([P, 512], F32, tag=f"fmst{ct}")
                    nc.sync.dma_start(
                        t[:], fm_d[ct * P:(ct + 1) * P, ch * 512:(ch + 1) * 512]
                    )
                    fm_tiles[ct] = t
                    t = fnst.tile([P, 512], F32, tag=f"fnst{ct}")
                    nc.sync.dma_start(
                        t[:], fn_d[ct * P:(ct + 1) * P, ch * 512:(ch + 1) * 512]
                    )
                    fn_tiles[ct] = t

                # round staging chunks to f32r. f_m rounds first: they gate
                # the PE transposes, so they must lead ACT's queue.
                fmr_tiles = {}
                for ct in range(CT):
                    r = fmrpool.tile([P, 512], F32R, tag=f"fmr{ct}")
                    nc.scalar.activation(r[:], fm_tiles[ct][:], CP)
                    fmr_tiles[ct] = r

                # f_n rounded copies stay resident (phase-3 GEMM rhs + the
                # f32r-transpose source); split DVE/ACT to balance load
                for ct in range(CT):
                    r = fnrpool.tile([P, 512], F32R, tag=f"fnr_{ct}_{ch}",
                                     name=f"fnr_{ct}_{ch}")
                    if ct < 2:
                        nc.vector.tensor_copy(r[:], fn_tiles[ct][:])
                    else:
                        nc.scalar.activation(r[:], fn_tiles[ct][:], CP)
                    fnr_res[ct][ch] = r

                for su in range(4):
                    s = ch * 4 + su
                    sl = slice(su * P, (su + 1) * P)
                    # transpose f_m subchunk -> [n128, c512] (f32r mode)
                    tpm = work.tile([P, C], F32R, tag="wk0", name="tpm")
                    for ct in range(CT):
                        nc.tensor.transpose(
                            tpm[:, ct * P:(ct + 1) * P],
                            fmr_tiles[ct][:, sl], ident_r[:]
                        )
                    fmT = ftpool.tile([P, C], F32R, tag="fmT")
                    nc.vector.tensor_copy(fmT[:], tpm[:])

                    # transpose f_n subchunk (f32r mode)
                    tpn = work.tile([P, C], F32R, tag="wk1", name="tpn")
                    for ct in range(CT):
                        nc.tensor.transpose(
                            tpn[:, ct * P:(ct + 1) * P],
                            fnr_res[ct][ch][:, sl], ident_r[:]
                        )
                    fnT = ftpool.tile([P, C], F32R, tag="fnT")
                    nc.vector.tensor_copy(fnT[:], tpn[:])

                    # Gram: G[a-tile, :] += fmT[:, a-tile].T @ fnT
                    for at in range(CT):
                        nc.tensor.matmul(
                            g_ps[at][:],
                            fmT[:, at * P:(at + 1) * P],
                            fnT[:],
                            start=(s == 0),
                            stop=(s == NSUB - 1),
                        )

            # ---------- weights (needed from phase 2; loaded during phase 1
            # DMA slack) + bout ----------
            def load_rounded(dram_rows, name):
                res = []
                for rt in range(CT):
                    st = wstage.tile([P, C], F32, tag=f"wnat{rt}")
                    nc.sync.dma_start(st[:], dram_rows[rt * P:(rt + 1) * P, :])
                    rs = wpool.tile([P, C], F32R, tag=f"{name}{rt}")
                    nc.scalar.activation(rs[:], st[:], CP)
                    res.append(rs)
                return res

            WqT = load_rounded(wqt_d, "wqT")      # WqT[a][., (h,i)]
            WkT = load_rounded(wkt_d, "wkT")      # WkT[b][., (h,j)]
            WoutT = load_rounded(woutt_d, "woT")  # WoutT[e][., o]
            Wv_r = load_rounded(wv_d, "wv")       # Wv rows (h,j), cols c

            bout_sb = []
            bview = bout_d.rearrange("(t p) -> t p", p=P)
            for ct in range(CT):
                bt = const.tile([P, 1], F32, tag=f"bout{ct}")
                nc.sync.dma_start(bt[:], bview[ct].unsqueeze(1))
                bout_sb.append(bt)

            G_sb = []
            for at in range(CT):
                g = small.tile([P, C], F32R, tag=f"G{at}")
                nc.vector.tensor_copy(g[:], g_ps[at][:])
                G_sb.append(g)

            # ---------- phase 2: logits, softmax, value mixing ------------
            # T2T[b, (h,i)] = sum_a G[a, b] * WqT[a, (h,i)]
            # (G natural as stationary -> transposed product for free)
            T2T_sb = []
            for bt in range(CT):
                ps = work.tile([P, C], F32, tag="wk1", name="t2tps")
                for at in range(CT):
                    nc.tensor.matmul(
                        ps[:],
                        G_sb[at][:, bt * P:(bt + 1) * P],
                        WqT[at][:],
                        start=(at == 0),
                        stop=(at == CT - 1),
                    )
                t = small.tile([P, C], F32R, tag=f"T2T_{bt}")
                nc.vector.tensor_copy(t[:], ps[:])
                T2T_sb.append(t)

            # Diagonal head-pair tiles of D^T = Wk @ T2T ; E^T = exp(scale*D^T)
            ET = []
            for jt in range(CT):
                sl = slice(jt * P, (jt + 1) * P)
                ps = work.tile([P, P], F32, tag="wk0", name="dps")
                for bt in range(CT):
                    nc.tensor.matmul(
                        ps[:], WkT[bt][:, sl], T2T_sb[bt][:, sl],
                        start=(bt == 0), stop=(bt == CT - 1),
                    )
                etmp = small.tile([P, P], F32, tag="etmp")
                nc.scalar.activation(etmp[:], ps[:], EXP, scale=SCALE)
                e = small.tile([P, P], F32R, tag=f"G{jt}", name=f"ET{jt}")
                # zero the cross-head blocks so full-width matmuls (SE,
                # rowsums) see exact per-head separation
                nc.vector.tensor_mul(e[:], etmp[:], dmask[:])
                ET.append(e)

            # rowsums r[(h,i)] = sum_j E_h[i, j]
            inv_sb = []
            for it in range(CT):
                rps = work.tile([P, 2], F32, tag="wk1", name="rps")
                nc.tensor.matmul(rps[:], ET[it][:], ones2_r[:], start=True,
                                 stop=True)
                inv = small.tile([P, 1], F32, tag=f"inv{it}")
                nc.vector.reciprocal(inv[:], rps[:, 0:1])
                inv_sb.append(inv)

            # SE_h = E_h @ Wv_h ; S = SE * inv_r (deferred softmax division)
            S_sb = []
            for it in range(CT):
                seps = work.tile([P, C], F32, tag="wk0", name="seps")
                nc.tensor.matmul(
                    seps[:], ET[it][:], Wv_r[it][:], start=True, stop=True,
                )
                s_t = small.tile([P, C], F32R, tag=f"S{it}", name=f"S{it}")
                nc.vector.tensor_scalar_mul(s_t[:], seps[:], inv_sb[it][:])
                S_sb.append(s_t)

            # M^T[c, o] = sum_e S[e][:, c] * WoutT[e][:, o]
            MT_sb = []
            for ct in range(CT):
                ps = work.tile([P, C], F32, tag="wk1", name="mtps")
                for et in range(CT):
                    nc.tensor.matmul(
                        ps[:],
                        S_sb[et][:, ct * P:(ct + 1) * P],
                        WoutT[et][:],
                        start=(et == 0),
                        stop=(et == CT - 1),
                    )
                t = small.tile([P, C], F32R, tag=f"T2T_{ct}", name=f"MT{ct}")
                nc.vector.tensor_copy(t[:], ps[:])
                MT_sb.append(t)

            # ---------- phase 3: out = M @ f_n + bout ----------------------
            for ch in range(NCH):
                fnr = [fnr_res[ct][ch] for ct in range(CT)]
                for ot in range(CT):
                    ps = gacc.tile([P, 512], F32, tag=f"g{ot}", name=f"ops{ot}")
                    for ct in range(CT):
                        nc.tensor.matmul(
                            ps[:],
                            MT_sb[ct][:, ot * P:(ot + 1) * P],
                            fnr[ct][:],
                            start=(ct == 0),
                            stop=(ct == CT - 1),
                        )
                    o = outst.tile([P, 512], F32, tag=f"out{ot}")
                    # ACT helps mid-stream; keep the last chunk all on DVE so
                    # the tail drains fast
                    if ot >= 2 and ch < NCH - 1:
                        nc.scalar.activation(o[:], ps[:], IDENT_FN,
                                             bias=bout_sb[ot][:])
                    else:
                        nc.vector.tensor_scalar_add(o[:], ps[:], bout_sb[ot][:])
                    nc.sync.dma_start(
                        out_d[ot * P:(ot + 1) * P, ch * 512:(ch + 1) * 512], o[:]
                    )

    nc.compile()
    return nc


def _get_nc():
    global _CACHED_NC
    if _CACHED_NC is None:
        _CACHED_NC = _build()
    return _CACHED_NC


def _get_runner():
    """Memoized PJRT runner: jax.jit-compiled once, reused across kernel()
    calls (run_bass_kernel_spmd rebuilds the jit closure every call, which
    forces a ~minute-long recompile)."""
    global _CACHED_RUNNER
    if _CACHED_RUNNER is not None:
        return _CACHED_RUNNER

    import jax
    from jax.sharding import Mesh, PartitionSpec
    from jax.experimental.shard_map import shard_map
    import concourse.mybir as mybir_
    from concourse.bass2jax import (
        _bass_exec_p,
        install_neuronx_cc_hook,
        partition_id_tensor,
    )

    nc = _get_nc()
    install_neuronx_cc_hook()

    partition_name = (
        nc.partition_id_tensor.name if nc.partition_id_tensor else None
    )
    in_names = []
    out_names = []
    out_avals = []
    out_shapes = []
    for alloc in nc.m.functions[0].allocations:
        if not isinstance(alloc, mybir_.MemoryLocationSet):
            continue
        name = alloc.memorylocations[0].name
        if alloc.kind == "ExternalInput":
            if name != partition_name:
                in_names.append(name)
        elif alloc.kind == "ExternalOutput":
            shape = tuple(alloc.tensor_shape)
            dtype = mybir_.dt.np(alloc.dtype)
            out_names.append(name)
            out_avals.append(jax.core.ShapedArray(shape, dtype))
            out_shapes.append((shape, dtype))
    n_params = len(in_names)
    n_outs = len(out_names)
    all_names = tuple(in_names + out_names)
    if partition_name is not None:
        all_names = all_names + (partition_name,)
    donate = tuple(range(n_params, n_params + n_outs))

    def _body(*args):
        operands = list(args)
        if partition_name is not None:
            operands.append(partition_id_tensor())
        outs = _bass_exec_p.bind(
            *operands,
            out_avals=tuple(out_avals),
            in_names=all_names,
            out_names=tuple(out_names),
            lowering_input_output_aliases=(),
            sim_require_finite=True,
            sim_require_nnan=True,
            nc=nc,
        )
        return tuple(outs)

    devices = jax.devices()[:B]
    mesh = Mesh(np.asarray(devices), ("core",))
    sharded = jax.jit(
        shard_map(
            _body,
            mesh=mesh,
            in_specs=(PartitionSpec("core"),) * (n_params + n_outs),
            out_specs=(PartitionSpec("core"),) * n_outs,
            check_rep=False,
        ),
        donate_argnums=donate,
        keep_unused=True,
    )

    def run(in_maps):
        concat_in = [
            np.concatenate([np.asarray(m[k]) for m in in_maps], axis=0)
            for k in in_names
        ]
        concat_zeros = [
            np.zeros((B * s[0], *s[1:]), dt) for (s, dt) in out_shapes
        ]
        out_arrs = sharded(*concat_in, *concat_zeros)
        return [
            {
                k: np.asarray(out_arrs[i]).reshape(B, *out_shapes[i][0])[c]
                for i, k in enumerate(out_names)
            }
            for c in range(B)
        ]

    _CACHED_RUNNER = run
    return run


def kernel(f_m, f_n, Wq, Wkv, Wout, bout, trace=False):
    f_m = np.ascontiguousarray(np.asarray(f_m, dtype=np.float32))
    f_n = np.ascontiguousarray(np.asarray(f_n, dtype=np.float32))
    Wq = np.ascontiguousarray(np.asarray(Wq, dtype=np.float32))
    Wkv = np.ascontiguousarray(np.asarray(Wkv, dtype=np.float32))
    Wout = np.ascontiguousarray(np.asarray(Wout, dtype=np.float32))
    bout = np.ascontiguousarray(np.asarray(bout, dtype=np.float32))

    b, c, h, w = f_m.shape
    nc = _get_nc()
    wqt = np.ascontiguousarray(Wq.T)
    wkt = np.ascontiguousarray(Wkv[:C].T)
    wv = np.ascontiguousarray(Wkv[C:])
    woutt = np.ascontiguousarray(Wout.T)
    in_maps = [
        {
            "f_m": f_m[i].reshape(C, NN),
            "f_n": f_n[i].reshape(C, NN),
            "WqT": wqt,
            "WkT": wkt,
            "Wv": wv,
            "WoutT": woutt,
            "bout": bout,
            "ident": _IDENT,
            "dmask": _DMASK,
            "ones2": _ONES2,
        }
        for i in range(b)
    ]
    if trace:
        res = run_bass_kernel_spmd(
            nc, in_maps, core_ids=list(range(B)), trace=True
        )
        kernel.last_results = res
        results = res.results
    else:
        results = _get_runner()(in_maps)
    return np.stack([r["out"].reshape(c, h, w) for r in results])
